# revision 17
# baseline (speedup 1.0000x reference)
"""Trainium2 Bass kernel for nn_LoRAAdapter (MoE-routed LoRA adapter).

Reference computation (B=4, S=2048, D=4096, OUT=4096, E=8, R=32, topk=2):
    routing_input = x[b, eof_index[b]]                     # [B, D]
    logits = routing_input @ route_w.T + noise * (softplus(routing_input @ noise_w.T) + eps)
    gates  = scatter(softmax(top2(logits)))                # [B, E]
    shared = x @ A_w.T                                     # [B, S, R]
    out    = einsum('bsr,eor,be->bso', shared, B_w, gates) * 2.0

Strategy:
  - Routing/gating runs on host (4 tokens' worth of math) and is folded into a
    per-batch effective B matrix:  Beff2[b] = 2.0 * sum_e gates[b,e] * B_w[e].
  - Data-parallel over tokens: 8192 tokens -> 8 cores x 1024 tokens
    (core i handles batch i//2, sequence half i%2).
  - All streamed tensors are bf16 (x in, out out): halves HBM traffic vs f32
    and runs the PE at 1 cycle/row instead of 4. PSUM accumulates f32; the
    measured end-to-end max-normalized rel err is ~4e-3 (budget 2e-2).
  - x is pre-transposed/blocked on host to [128, NBLK, G, 8*BLK] so each
    (block, group) is one big contiguous DMA (8 KiB per partition line).
  - Per core: sharedT[R, BLK] = sum_c AT_c.T @ xT_c on PE, then
    out[tok, OUT] = sharedT.T @ B2T on PE, PSUM -> SBUF bf16 copies spread
    across Vector/Scalar/GpSimd engines, DMA out.
"""

import numpy as np
import ml_dtypes

import concourse.bass as bass
import concourse.mybir as mybir
import concourse.tile as tile
import bass_rust
from concourse.bass_utils import run_bass_kernel_spmd

B, S, D, OUT, E, R = 4, 2048, 4096, 4096, 8, 32
TOPK = 2
NOISE_EPS = 0.01
SCALING = 2.0
N_CORES = 8
TOK = (B * S) // N_CORES          # 1024 tokens per core
BLK = 512                         # token block (mm1 moving dim)
NBLK = TOK // BLK
DCH = D // 128                    # 32 contraction chunks of 128
G = 4                             # x DMA groups per block (8 chunks each)
CPG = DCH // G
OCH = OUT // 512                  # 8 output column chunks

BF16 = ml_dtypes.bfloat16

_MAXW = 1  # this container's walrus rejects >1 sync wait per instruction


def _legalize_waits(nc):
    """Split instructions carrying >_MAXW sem waits into preceding
    same-engine nops (the kernel-tail drain waits on the whole clock).

    Two passes: nop creation appends the new instruction to the *current*
    basic block regardless of which block we are fixing, so snapshot every
    block first and rebuild each list from its own snapshot (stray appends
    then drop out naturally)."""
    snapshots = []
    for f in nc.m.functions:
        for bb in f.blocks:
            snapshots.append((bb, list(bb.instructions)))

    nops_for: dict[str, list] = {}
    for _, insts in snapshots:
        for inst in insts:
            si = inst.sync_info
            if si and si.on_wait and len(si.on_wait) > _MAXW:
                waits = list(si.on_wait)
                eng = nc.engines[inst.engine]
                extras = []
                for k in range(0, len(waits) - _MAXW, _MAXW):
                    nop = eng.nop(hint="wait_split", nofuse=True).ins
                    nop.sync_info = bass_rust.SyncInfo(
                        on_wait=waits[k : k + _MAXW], on_update=[]
                    )
                    extras.append(nop)
                si.on_wait = waits[len(waits) - _MAXW :]
                inst.sync_info = si
                nops_for[inst.name] = extras

    if not nops_for:
        return
    for bb, insts in snapshots:
        rebuilt = []
        for inst in insts:
            rebuilt.extend(nops_for.get(inst.name, ()))
            rebuilt.append(inst)
        bb.instructions = rebuilt


def build_bass():
    f32 = mybir.dt.float32
    bf16 = mybir.dt.bfloat16
    nc = bass.Bass()
    xT = nc.dram_tensor("xT", [128, NBLK, G, CPG * BLK], bf16, kind="ExternalInput")
    AT = nc.dram_tensor("AT", [128, DCH * R], bf16, kind="ExternalInput")
    B2T = nc.dram_tensor("B2T", [R, OUT], bf16, kind="ExternalInput")
    out = nc.dram_tensor("out", [TOK, OUT], bf16, kind="ExternalOutput")

    with tile.TileContext(nc) as tc:
        with (
            tc.tile_pool(name="const", bufs=1) as cpool,
            tc.tile_pool(name="xs", bufs=1) as xpool,
            tc.tile_pool(name="sh", bufs=2) as shpool,
            tc.tile_pool(name="ob", bufs=5) as opool,
            tc.tile_pool(name="psA", bufs=2, space="PSUM") as psa,
            tc.tile_pool(name="psB", bufs=4, space="PSUM") as psb,
        ):
            at_t = cpool.tile([128, DCH * R], bf16)
            nc.sync.dma_start(at_t[:], AT[:, :])
            b2_t = cpool.tile([R, OUT], bf16)

            # PSUM->SBUF copies round-robin across the two engines that can
            # read PSUM (GpSimd cannot on TRN2) so neither becomes the
            # bottleneck at the DMA-bound target.
            def copy_ops(nc):
                return [nc.vector.tensor_copy, nc.scalar.copy]

            # All x loads issued upfront: keeps the load stream off the
            # Sync queue's critical path (out-store dma_starts wait on
            # compute; anything emitted after them would head-of-line
            # block).
            xg = {}
            for blk in range(NBLK):
                for g in range(G):
                    xt = xpool.tile([128, CPG * BLK], bf16, tag=f"xg{blk}_{g}")
                    if blk == 0 and g == 0:
                        # split the very first group into per-chunk DMAs so
                        # the first matmul's data lands ~3x earlier (DMA
                        # completion is per-descriptor)
                        for j in range(CPG):
                            nc.sync.dma_start(
                                xt[:, j * BLK : (j + 1) * BLK],
                                xT[:, 0, 0, j * BLK : (j + 1) * BLK],
                            )
                    else:
                        nc.sync.dma_start(xt[:], xT[:, blk, g, :])
                    xg[blk, g] = xt
                if blk == 0:
                    # B2 isn't needed until mm2 (~half-way in); issuing it
                    # here keeps block 0's x groups at the queue head
                    nc.sync.dma_start(b2_t[:], B2T[:, :])

            # ---- mm1(b0): sharedT[R, BLK] = sum_c AT_c.T @ xT_c ----
            sh = {}

            def mm1(blk):
                ps_sh = psa.tile([R, BLK], f32, tag="ps_sh")
                for c in range(DCH):
                    g, j = divmod(c, CPG)
                    nc.tensor.matmul(
                        ps_sh[:],
                        lhsT=at_t[:, c * R : (c + 1) * R],
                        rhs=xg[blk, g][:, j * BLK : (j + 1) * BLK],
                        start=(c == 0),
                        stop=(c == DCH - 1),
                    )
                sh_sb = shpool.tile([R, BLK], bf16, tag="sh")
                nc.vector.tensor_copy(sh_sb[:], ps_sh[:])
                sh[blk] = sh_sb

            ci = 0
            ot_tiles = {}

            def mm2_mm(blk, t4, o, split_copy=False):
                nonlocal ci
                if (blk, t4) not in ot_tiles:
                    ot_tiles[blk, t4] = opool.tile(
                        [128, OUT], bf16, tag="ot", name=f"ot{blk}_{t4}"
                    )
                ot = ot_tiles[blk, t4]
                ps_o = psb.tile([128, 512], f32, tag="ps_o")
                nc.tensor.matmul(
                    ps_o[:],
                    lhsT=sh[blk][:, t4 * 128 : (t4 + 1) * 128],
                    rhs=b2_t[:, o * 512 : (o + 1) * 512],
                    start=True,
                    stop=True,
                )
                oc = ot[:, o * 512 : (o + 1) * 512]
                if split_copy:
                    # tail: halve the copy across both engines so the final
                    # store isn't gated on one 690ns copy
                    nc.vector.tensor_copy(oc[:, 0:256], ps_o[:, 0:256])
                    nc.scalar.copy(oc[:, 256:512], ps_o[:, 256:512])
                else:
                    op = copy_ops(nc)[ci % 2]
                    ci += 1
                    op(oc, ps_o[:])

            def store(blk, t4, o_hi, parts):
                # store [0, o_hi*512) of tile (blk, t4) in `parts` DMAs
                t0 = blk * BLK
                ot = ot_tiles[blk, t4]
                w = o_hi * 512 // parts
                for p in range(parts):
                    nc.sync.dma_start(
                        out[t0 + t4 * 128 : t0 + (t4 + 1) * 128, p * w : (p + 1) * w],
                        ot[:, p * w : (p + 1) * w],
                    )

            # Phase 1 is x-DMA-feed-bound with ~7us of PE idle: fill it by
            # interleaving the first 16 mm2(b0) matmuls between mm1(b1)
            # chunks.  Their out-stores are deferred to phase 2 so they
            # don't steal x-load DMA bandwidth.
            mm1(0)
            inj = [(0, t4, o) for t4 in range(2) for o in range(OCH)]
            ps_sh1 = psa.tile([R, BLK], f32, tag="ps_sh")
            for c in range(DCH):
                g, j = divmod(c, CPG)
                nc.tensor.matmul(
                    ps_sh1[:],
                    lhsT=at_t[:, c * R : (c + 1) * R],
                    rhs=xg[1, g][:, j * BLK : (j + 1) * BLK],
                    start=(c == 0),
                    stop=(c == DCH - 1),
                )
                if c % 2 == 1 and inj:
                    mm2_mm(*inj.pop(0))
            sh_sb1 = shpool.tile([R, BLK], bf16, tag="sh")
            nc.vector.tensor_copy(sh_sb1[:], ps_sh1[:])
            sh[1] = sh_sb1

            # Phase 2: stores of the pre-computed tiles first (DMA starts
            # on them immediately), then the remaining mm2 work.
            store(0, 0, OCH, 2)
            store(0, 1, OCH, 2)
            rest = [(0, t4, o) for t4 in range(2, 4) for o in range(OCH)] + [
                (1, t4, o) for t4 in range(4) for o in range(OCH)
            ]
            for blk, t4, o in rest:
                last = (blk, t4) == (1, 3)
                mm2_mm(blk, t4, o, split_copy=last and o >= OCH - 2)
                part = OCH // 4 if last else OCH // 2
                if (o + 1) % part == 0:
                    hi = o + 1
                    t0 = blk * BLK
                    lo = (o + 1 - part) * 512
                    nc.sync.dma_start(
                        out[t0 + t4 * 128 : t0 + (t4 + 1) * 128, lo : lo + part * 512],
                        ot_tiles[blk, t4][:, lo : lo + part * 512],
                    )
    _legalize_waits(nc)
    return nc


_NC_CACHE = {}


def _get_nc():
    if "nc" not in _NC_CACHE:
        _NC_CACHE["nc"] = build_bass()
    return _NC_CACHE["nc"]


def _softplus(v):
    return np.logaddexp(0.0, v)


def _host_prep(x, eof_index, noise, A_w, B_w, route_w, noise_w):
    """Routing + gating on host; returns per-core input maps."""
    x = np.asarray(x, dtype=np.float32)
    eof = np.asarray(eof_index).astype(np.int64)
    noise = np.asarray(noise, dtype=np.float32)
    A_w = np.asarray(A_w, dtype=np.float32)
    B_w = np.asarray(B_w, dtype=np.float32)
    route_w = np.asarray(route_w, dtype=np.float32)
    noise_w = np.asarray(noise_w, dtype=np.float32)

    rows = np.arange(B)
    routing_input = x[rows, eof]                                  # [B, D]
    clean = routing_input @ route_w.T                             # [B, E]
    stddev = _softplus(routing_input @ noise_w.T) + NOISE_EPS
    logits = clean + noise * stddev
    top_idx = np.argsort(-logits, axis=-1, kind="stable")[:, :TOPK]
    top_vals = np.take_along_axis(logits, top_idx, axis=-1)
    m = top_vals.max(axis=-1, keepdims=True)
    ex = np.exp(top_vals - m)
    top_gates = (ex / ex.sum(axis=-1, keepdims=True)).astype(np.float32)
    gates = np.zeros((B, E), np.float32)
    np.put_along_axis(gates, top_idx, top_gates, axis=-1)

    # Beff2[b] = SCALING * sum_e gates[b,e] * B_w[e]   -> [B, OUT, R]
    beff2 = SCALING * np.einsum("be,eor->bor", gates, B_w)

    # AT layout [128, DCH, R]: at[p, c, r] = A_w[r, c*128+p]
    at = np.ascontiguousarray(
        A_w.reshape(R, DCH, 128).transpose(2, 1, 0).reshape(128, DCH * R)
    ).astype(BF16)

    in_maps = []
    for i in range(N_CORES):
        b = i * TOK // S
        t0 = i * TOK - b * S
        xc = x[b, t0 : t0 + TOK, :]                               # [TOK, D]
        # [128, NBLK, DCH, BLK]: xT[p, blk, c, t] = xc[blk*BLK+t, c*128+p]
        xT_i = np.ascontiguousarray(
            xc.reshape(NBLK, BLK, DCH, 128).transpose(3, 0, 2, 1)
        ).astype(BF16).reshape(128, NBLK, G, CPG * BLK)
        b2t_i = np.ascontiguousarray(beff2[b].T).astype(BF16)     # [R, OUT]
        in_maps.append({"xT": xT_i, "AT": at, "B2T": b2t_i})
    return in_maps


def _run(in_maps, trace=False, **kw):
    nc = _get_nc()
    return run_bass_kernel_spmd(
        nc, in_maps, core_ids=list(range(N_CORES)), trace=trace, **kw
    )


def kernel(x, eof_index, noise, A_w, B_w, route_w, noise_w):
    in_maps = _host_prep(x, eof_index, noise, A_w, B_w, route_w, noise_w)
    res = _run(in_maps)
    out = np.empty((B, S, OUT), np.float32)
    for i in range(N_CORES):
        b = i * TOK // S
        t0 = i * TOK - b * S
        out[b, t0 : t0 + TOK, :] = np.asarray(res.results[i]["out"]).astype(
            np.float32
        )
    return out


# revision 18
# speedup vs baseline: 1.0493x; 1.0493x over previous
"""Trainium2 Bass kernel for nn_LoRAAdapter (MoE-routed LoRA adapter).

Reference computation (B=4, S=2048, D=4096, OUT=4096, E=8, R=32, topk=2):
    routing_input = x[b, eof_index[b]]                     # [B, D]
    logits = routing_input @ route_w.T + noise * (softplus(routing_input @ noise_w.T) + eps)
    gates  = scatter(softmax(top2(logits)))                # [B, E]
    shared = x @ A_w.T                                     # [B, S, R]
    out    = einsum('bsr,eor,be->bso', shared, B_w, gates) * 2.0

Strategy:
  - Routing/gating runs on host (4 tokens' worth of math) and is folded into a
    per-batch effective B matrix:  Beff2[b] = 2.0 * sum_e gates[b,e] * B_w[e].
  - Data-parallel over tokens: 8192 tokens -> 8 cores x 1024 tokens
    (core i handles batch i//2, sequence half i%2).
  - All streamed tensors are bf16 (x in, out out): halves HBM traffic vs f32
    and runs the PE at 1 cycle/row instead of 4. PSUM accumulates f32; the
    measured end-to-end max-normalized rel err is ~4e-3 (budget 2e-2).
  - x is pre-transposed/blocked on host to [128, NBLK, G, 8*BLK] so each
    (block, group) is one big contiguous DMA (8 KiB per partition line).
  - Per core: sharedT[R, BLK] = sum_c AT_c.T @ xT_c on PE, then
    out[tok, OUT] = sharedT.T @ B2T on PE, PSUM -> SBUF bf16 copies spread
    across Vector/Scalar/GpSimd engines, DMA out.
"""

import numpy as np
import ml_dtypes

import concourse.bass as bass
import concourse.mybir as mybir
import concourse.tile as tile
import bass_rust
from concourse.bass_utils import run_bass_kernel_spmd

B, S, D, OUT, E, R = 4, 2048, 4096, 4096, 8, 32
TOPK = 2
NOISE_EPS = 0.01
SCALING = 2.0
N_CORES = 8
TOK = (B * S) // N_CORES          # 1024 tokens per core
BLK = 512                         # token block (mm1 moving dim)
NBLK = TOK // BLK
DCH = D // 128                    # 32 contraction chunks of 128
G = 4                             # x DMA groups per block (8 chunks each)
CPG = DCH // G
OCH = OUT // 512                  # 8 output column chunks

BF16 = ml_dtypes.bfloat16

_MAXW = 1  # this container's walrus rejects >1 sync wait per instruction


def _legalize_waits(nc):
    """Split instructions carrying >_MAXW sem waits into preceding
    same-engine nops (the kernel-tail drain waits on the whole clock).

    Two passes: nop creation appends the new instruction to the *current*
    basic block regardless of which block we are fixing, so snapshot every
    block first and rebuild each list from its own snapshot (stray appends
    then drop out naturally)."""
    snapshots = []
    for f in nc.m.functions:
        for bb in f.blocks:
            snapshots.append((bb, list(bb.instructions)))

    nops_for: dict[str, list] = {}
    for _, insts in snapshots:
        for inst in insts:
            si = inst.sync_info
            if si and si.on_wait and len(si.on_wait) > _MAXW:
                waits = list(si.on_wait)
                eng = nc.engines[inst.engine]
                extras = []
                for k in range(0, len(waits) - _MAXW, _MAXW):
                    nop = eng.nop(hint="wait_split", nofuse=True).ins
                    nop.sync_info = bass_rust.SyncInfo(
                        on_wait=waits[k : k + _MAXW], on_update=[]
                    )
                    extras.append(nop)
                si.on_wait = waits[len(waits) - _MAXW :]
                inst.sync_info = si
                nops_for[inst.name] = extras

    if not nops_for:
        return
    for bb, insts in snapshots:
        rebuilt = []
        for inst in insts:
            rebuilt.extend(nops_for.get(inst.name, ()))
            rebuilt.append(inst)
        bb.instructions = rebuilt


def build_bass():
    f32 = mybir.dt.float32
    bf16 = mybir.dt.bfloat16
    nc = bass.Bass()
    xT = nc.dram_tensor("xT", [128, NBLK, G, CPG * BLK], bf16, kind="ExternalInput")
    AT = nc.dram_tensor("AT", [128, DCH * R], bf16, kind="ExternalInput")
    B2T = nc.dram_tensor("B2T", [R, OUT], bf16, kind="ExternalInput")
    out = nc.dram_tensor("out", [TOK, OUT], bf16, kind="ExternalOutput")

    with tile.TileContext(nc) as tc:
        with (
            tc.tile_pool(name="const", bufs=1) as cpool,
            tc.tile_pool(name="xs", bufs=1) as xpool,
            tc.tile_pool(name="sh", bufs=2) as shpool,
            tc.tile_pool(name="ob", bufs=5) as opool,
            tc.tile_pool(name="psA", bufs=2, space="PSUM") as psa,
            tc.tile_pool(name="psB", bufs=4, space="PSUM") as psb,
        ):
            at_t = cpool.tile([128, DCH * R], bf16)
            nc.sync.dma_start(at_t[:], AT[:, :])
            b2_t = cpool.tile([R, OUT], bf16)

            # PSUM->SBUF copies round-robin across the two engines that can
            # read PSUM (GpSimd cannot on TRN2) so neither becomes the
            # bottleneck at the DMA-bound target.
            def copy_ops(nc):
                return [nc.vector.tensor_copy, nc.scalar.copy]

            # All x loads issued upfront: keeps the load stream off the
            # Sync queue's critical path (out-store dma_starts wait on
            # compute; anything emitted after them would head-of-line
            # block).
            xg = {}
            for blk in range(NBLK):
                for g in range(G):
                    xt = xpool.tile([128, CPG * BLK], bf16, tag=f"xg{blk}_{g}")
                    if blk == 0 and g == 0:
                        # split the very first group into per-chunk DMAs so
                        # the first matmul's data lands ~3x earlier (DMA
                        # completion is per-descriptor)
                        for j in range(CPG):
                            nc.sync.dma_start(
                                xt[:, j * BLK : (j + 1) * BLK],
                                xT[:, 0, 0, j * BLK : (j + 1) * BLK],
                            )
                    else:
                        nc.sync.dma_start(xt[:], xT[:, blk, g, :])
                    xg[blk, g] = xt
                if blk == 0:
                    # B2 isn't needed until mm2 (~half-way in); issuing it
                    # here keeps block 0's x groups at the queue head
                    nc.sync.dma_start(b2_t[:], B2T[:, :])

            # ---- mm1(b0): sharedT[R, BLK] = sum_c AT_c.T @ xT_c ----
            sh = {}

            def mm1(blk):
                ps_sh = psa.tile([R, BLK], f32, tag="ps_sh")
                for c in range(DCH):
                    g, j = divmod(c, CPG)
                    nc.tensor.matmul(
                        ps_sh[:],
                        lhsT=at_t[:, c * R : (c + 1) * R],
                        rhs=xg[blk, g][:, j * BLK : (j + 1) * BLK],
                        start=(c == 0),
                        stop=(c == DCH - 1),
                    )
                sh_sb = shpool.tile([R, BLK], bf16, tag="sh")
                nc.vector.tensor_copy(sh_sb[:], ps_sh[:])
                sh[blk] = sh_sb

            ci = 0
            ot_tiles = {}

            def mm2_mm(blk, t4, o, split_copy=False):
                nonlocal ci
                if (blk, t4) not in ot_tiles:
                    ot_tiles[blk, t4] = opool.tile(
                        [128, OUT], bf16, tag="ot", name=f"ot{blk}_{t4}"
                    )
                ot = ot_tiles[blk, t4]
                ps_o = psb.tile([128, 512], f32, tag="ps_o")
                nc.tensor.matmul(
                    ps_o[:],
                    lhsT=sh[blk][:, t4 * 128 : (t4 + 1) * 128],
                    rhs=b2_t[:, o * 512 : (o + 1) * 512],
                    start=True,
                    stop=True,
                )
                oc = ot[:, o * 512 : (o + 1) * 512]
                if split_copy:
                    # tail: halve the copy across both engines so the final
                    # store isn't gated on one 690ns copy
                    nc.vector.tensor_copy(oc[:, 0:256], ps_o[:, 0:256])
                    nc.scalar.copy(oc[:, 256:512], ps_o[:, 256:512])
                else:
                    op = copy_ops(nc)[ci % 2]
                    ci += 1
                    op(oc, ps_o[:])

            def store(blk, t4, o_hi, parts):
                # store [0, o_hi*512) of tile (blk, t4) in `parts` DMAs
                t0 = blk * BLK
                ot = ot_tiles[blk, t4]
                w = o_hi * 512 // parts
                for p in range(parts):
                    nc.sync.dma_start(
                        out[t0 + t4 * 128 : t0 + (t4 + 1) * 128, p * w : (p + 1) * w],
                        ot[:, p * w : (p + 1) * w],
                    )

            # Phase 1 is x-DMA-feed-bound with ~7us of PE idle: fill it by
            # interleaving the first 16 mm2(b0) matmuls between mm1(b1)
            # chunks.  Their out-stores are deferred to phase 2 so they
            # don't steal x-load DMA bandwidth.
            mm1(0)
            mm1(1)

            # Phase 2.  The HW activity governor clamps the PE to 50% util
            # after ~20us of full-duty streaming; pace the mm2 stream with
            # nops (~7% duty relief) to try to stay under the trigger.
            rest = [
                (blk, t4, o)
                for blk in range(NBLK)
                for t4 in range(4)
                for o in range(OCH)
            ]
            for blk, t4, o in rest:
                last = (blk, t4) == (1, 3)
                mm2_mm(blk, t4, o, split_copy=last and o >= OCH - 2)
                nc.tensor.nop(hint="pace", nofuse=True)
                nc.tensor.nop(hint="pace", nofuse=True)
                part = OCH // 4 if last else OCH // 2
                if (o + 1) % part == 0:
                    t0 = blk * BLK
                    lo = (o + 1 - part) * 512
                    nc.sync.dma_start(
                        out[t0 + t4 * 128 : t0 + (t4 + 1) * 128, lo : lo + part * 512],
                        ot_tiles[blk, t4][:, lo : lo + part * 512],
                    )
    _legalize_waits(nc)
    return nc


_NC_CACHE = {}


def _get_nc():
    if "nc" not in _NC_CACHE:
        _NC_CACHE["nc"] = build_bass()
    return _NC_CACHE["nc"]


def _softplus(v):
    return np.logaddexp(0.0, v)


def _host_prep(x, eof_index, noise, A_w, B_w, route_w, noise_w):
    """Routing + gating on host; returns per-core input maps."""
    x = np.asarray(x, dtype=np.float32)
    eof = np.asarray(eof_index).astype(np.int64)
    noise = np.asarray(noise, dtype=np.float32)
    A_w = np.asarray(A_w, dtype=np.float32)
    B_w = np.asarray(B_w, dtype=np.float32)
    route_w = np.asarray(route_w, dtype=np.float32)
    noise_w = np.asarray(noise_w, dtype=np.float32)

    rows = np.arange(B)
    routing_input = x[rows, eof]                                  # [B, D]
    clean = routing_input @ route_w.T                             # [B, E]
    stddev = _softplus(routing_input @ noise_w.T) + NOISE_EPS
    logits = clean + noise * stddev
    top_idx = np.argsort(-logits, axis=-1, kind="stable")[:, :TOPK]
    top_vals = np.take_along_axis(logits, top_idx, axis=-1)
    m = top_vals.max(axis=-1, keepdims=True)
    ex = np.exp(top_vals - m)
    top_gates = (ex / ex.sum(axis=-1, keepdims=True)).astype(np.float32)
    gates = np.zeros((B, E), np.float32)
    np.put_along_axis(gates, top_idx, top_gates, axis=-1)

    # Beff2[b] = SCALING * sum_e gates[b,e] * B_w[e]   -> [B, OUT, R]
    beff2 = SCALING * np.einsum("be,eor->bor", gates, B_w)

    # AT layout [128, DCH, R]: at[p, c, r] = A_w[r, c*128+p]
    at = np.ascontiguousarray(
        A_w.reshape(R, DCH, 128).transpose(2, 1, 0).reshape(128, DCH * R)
    ).astype(BF16)

    in_maps = []
    for i in range(N_CORES):
        b = i * TOK // S
        t0 = i * TOK - b * S
        xc = x[b, t0 : t0 + TOK, :]                               # [TOK, D]
        # [128, NBLK, DCH, BLK]: xT[p, blk, c, t] = xc[blk*BLK+t, c*128+p]
        xT_i = np.ascontiguousarray(
            xc.reshape(NBLK, BLK, DCH, 128).transpose(3, 0, 2, 1)
        ).astype(BF16).reshape(128, NBLK, G, CPG * BLK)
        b2t_i = np.ascontiguousarray(beff2[b].T).astype(BF16)     # [R, OUT]
        in_maps.append({"xT": xT_i, "AT": at, "B2T": b2t_i})
    return in_maps


def _run(in_maps, trace=False, **kw):
    nc = _get_nc()
    return run_bass_kernel_spmd(
        nc, in_maps, core_ids=list(range(N_CORES)), trace=trace, **kw
    )


def kernel(x, eof_index, noise, A_w, B_w, route_w, noise_w):
    in_maps = _host_prep(x, eof_index, noise, A_w, B_w, route_w, noise_w)
    res = _run(in_maps)
    out = np.empty((B, S, OUT), np.float32)
    for i in range(N_CORES):
        b = i * TOK // S
        t0 = i * TOK - b * S
        out[b, t0 : t0 + TOK, :] = np.asarray(res.results[i]["out"]).astype(
            np.float32
        )
    return out


# revision 19
# speedup vs baseline: 1.1594x; 1.1050x over previous
"""Trainium2 Bass kernel for nn_LoRAAdapter (MoE-routed LoRA adapter).

Reference computation (B=4, S=2048, D=4096, OUT=4096, E=8, R=32, topk=2):
    routing_input = x[b, eof_index[b]]                     # [B, D]
    logits = routing_input @ route_w.T + noise * (softplus(routing_input @ noise_w.T) + eps)
    gates  = scatter(softmax(top2(logits)))                # [B, E]
    shared = x @ A_w.T                                     # [B, S, R]
    out    = einsum('bsr,eor,be->bso', shared, B_w, gates) * 2.0

Strategy:
  - Routing/gating runs on host (4 tokens' worth of math) and is folded into a
    per-batch effective B matrix:  Beff2[b] = 2.0 * sum_e gates[b,e] * B_w[e].
  - Data-parallel over tokens: 8192 tokens -> 8 cores x 1024 tokens
    (core i handles batch i//2, sequence half i%2).
  - All streamed tensors are bf16 (x in, out out): halves HBM traffic vs f32
    and runs the PE at 1 cycle/row instead of 4. PSUM accumulates f32; the
    measured end-to-end max-normalized rel err is ~4e-3 (budget 2e-2).
  - x is pre-transposed/blocked on host to [128, NBLK, G, 8*BLK] so each
    (block, group) is one big contiguous DMA (8 KiB per partition line).
  - Per core: sharedT[R, BLK] = sum_c AT_c.T @ xT_c on PE, then
    out[tok, OUT] = sharedT.T @ B2T on PE, PSUM -> SBUF bf16 copies spread
    across Vector/Scalar/GpSimd engines, DMA out.
"""

import numpy as np
import ml_dtypes

import concourse.bass as bass
import concourse.mybir as mybir
import concourse.tile as tile
import bass_rust
from concourse.bass_utils import run_bass_kernel_spmd

B, S, D, OUT, E, R = 4, 2048, 4096, 4096, 8, 32
TOPK = 2
NOISE_EPS = 0.01
SCALING = 2.0
N_CORES = 8
TOK = (B * S) // N_CORES          # 1024 tokens per core
BLK = 512                         # token block (mm1 moving dim)
NBLK = TOK // BLK
DCH = D // 128                    # 32 contraction chunks of 128
G = 4                             # x DMA groups per block (8 chunks each)
CPG = DCH // G
OCH = OUT // 512                  # 8 output column chunks

BF16 = ml_dtypes.bfloat16

_MAXW = 1  # this container's walrus rejects >1 sync wait per instruction


def _legalize_waits(nc):
    """Split instructions carrying >_MAXW sem waits into preceding
    same-engine nops (the kernel-tail drain waits on the whole clock).

    Two passes: nop creation appends the new instruction to the *current*
    basic block regardless of which block we are fixing, so snapshot every
    block first and rebuild each list from its own snapshot (stray appends
    then drop out naturally)."""
    snapshots = []
    for f in nc.m.functions:
        for bb in f.blocks:
            snapshots.append((bb, list(bb.instructions)))

    nops_for: dict[str, list] = {}
    for _, insts in snapshots:
        for inst in insts:
            si = inst.sync_info
            if si and si.on_wait and len(si.on_wait) > _MAXW:
                waits = list(si.on_wait)
                eng = nc.engines[inst.engine]
                extras = []
                for k in range(0, len(waits) - _MAXW, _MAXW):
                    nop = eng.nop(hint="wait_split", nofuse=True).ins
                    nop.sync_info = bass_rust.SyncInfo(
                        on_wait=waits[k : k + _MAXW], on_update=[]
                    )
                    extras.append(nop)
                si.on_wait = waits[len(waits) - _MAXW :]
                inst.sync_info = si
                nops_for[inst.name] = extras

    if not nops_for:
        return
    for bb, insts in snapshots:
        rebuilt = []
        for inst in insts:
            rebuilt.extend(nops_for.get(inst.name, ()))
            rebuilt.append(inst)
        bb.instructions = rebuilt


def build_bass():
    f32 = mybir.dt.float32
    bf16 = mybir.dt.bfloat16
    nc = bass.Bass()
    xT = nc.dram_tensor("xT", [128, NBLK, G, CPG * BLK], bf16, kind="ExternalInput")
    AT = nc.dram_tensor("AT", [128, DCH * R], bf16, kind="ExternalInput")
    B2T = nc.dram_tensor("B2T", [R, OUT], bf16, kind="ExternalInput")
    out = nc.dram_tensor("out", [TOK, OUT], bf16, kind="ExternalOutput")

    with tile.TileContext(nc) as tc:
        with (
            tc.tile_pool(name="const", bufs=1) as cpool,
            tc.tile_pool(name="xs", bufs=1) as xpool,
            tc.tile_pool(name="sh", bufs=2) as shpool,
            tc.tile_pool(name="ob", bufs=5) as opool,
            tc.tile_pool(name="psA", bufs=2, space="PSUM") as psa,
            tc.tile_pool(name="psB", bufs=4, space="PSUM") as psb,
        ):
            at_t = cpool.tile([128, DCH * R], bf16)
            nc.sync.dma_start(at_t[:], AT[:, :])
            b2_t = cpool.tile([R, OUT], bf16)

            # PSUM->SBUF copies round-robin across the two engines that can
            # read PSUM (GpSimd cannot on TRN2) so neither becomes the
            # bottleneck at the DMA-bound target.
            def copy_ops(nc):
                return [nc.vector.tensor_copy, nc.scalar.copy]

            # All x loads issued upfront: keeps the load stream off the
            # Sync queue's critical path (out-store dma_starts wait on
            # compute; anything emitted after them would head-of-line
            # block).
            xg = {}
            for blk in range(NBLK):
                for g in range(G):
                    xt = xpool.tile([128, CPG * BLK], bf16, tag=f"xg{blk}_{g}")
                    if blk == 0 and g == 0:
                        # split the very first group into per-chunk DMAs so
                        # the first matmul's data lands ~3x earlier (DMA
                        # completion is per-descriptor)
                        for j in range(CPG):
                            nc.sync.dma_start(
                                xt[:, j * BLK : (j + 1) * BLK],
                                xT[:, 0, 0, j * BLK : (j + 1) * BLK],
                            )
                    else:
                        nc.sync.dma_start(xt[:], xT[:, blk, g, :])
                    xg[blk, g] = xt
                if blk == 0:
                    # B2 isn't needed until mm2 (~half-way in); issuing it
                    # here keeps block 0's x groups at the queue head
                    nc.sync.dma_start(b2_t[:], B2T[:, :])

            # ---- mm1(b0): sharedT[R, BLK] = sum_c AT_c.T @ xT_c ----
            sh = {}

            def mm1(blk):
                ps_sh = psa.tile([R, BLK], f32, tag="ps_sh")
                for c in range(DCH):
                    g, j = divmod(c, CPG)
                    nc.tensor.matmul(
                        ps_sh[:],
                        lhsT=at_t[:, c * R : (c + 1) * R],
                        rhs=xg[blk, g][:, j * BLK : (j + 1) * BLK],
                        start=(c == 0),
                        stop=(c == DCH - 1),
                    )
                sh_sb = shpool.tile([R, BLK], bf16, tag="sh")
                nc.vector.tensor_copy(sh_sb[:], ps_sh[:])
                sh[blk] = sh_sb

            ci = 0
            ot_tiles = {}

            def mm2_mm(blk, t4, o, split_copy=False):
                nonlocal ci
                if (blk, t4) not in ot_tiles:
                    ot_tiles[blk, t4] = opool.tile(
                        [128, OUT], bf16, tag="ot", name=f"ot{blk}_{t4}"
                    )
                ot = ot_tiles[blk, t4]
                ps_o = psb.tile([128, 512], f32, tag="ps_o")
                nc.tensor.matmul(
                    ps_o[:],
                    lhsT=sh[blk][:, t4 * 128 : (t4 + 1) * 128],
                    rhs=b2_t[:, o * 512 : (o + 1) * 512],
                    start=True,
                    stop=True,
                )
                oc = ot[:, o * 512 : (o + 1) * 512]
                if split_copy:
                    # tail: halve the copy across both engines so the final
                    # store isn't gated on one 690ns copy
                    nc.vector.tensor_copy(oc[:, 0:256], ps_o[:, 0:256])
                    nc.scalar.copy(oc[:, 256:512], ps_o[:, 256:512])
                else:
                    op = copy_ops(nc)[ci % 2]
                    ci += 1
                    op(oc, ps_o[:])

            def store(blk, t4, o_hi, parts):
                # store [0, o_hi*512) of tile (blk, t4) in `parts` DMAs
                t0 = blk * BLK
                ot = ot_tiles[blk, t4]
                w = o_hi * 512 // parts
                for p in range(parts):
                    nc.sync.dma_start(
                        out[t0 + t4 * 128 : t0 + (t4 + 1) * 128, p * w : (p + 1) * w],
                        ot[:, p * w : (p + 1) * w],
                    )

            # Phase 1 is x-DMA-feed-bound with ~7us of PE idle: fill it by
            # interleaving the first 16 mm2(b0) matmuls between mm1(b1)
            # chunks.  Their out-stores are deferred to phase 2 so they
            # don't steal x-load DMA bandwidth.
            mm1(0)
            mm1(1)

            # Phase 2.  The HW activity governor clamps the PE to 50% util
            # after ~20us of full-duty streaming; pace the mm2 stream with
            # nops (~7% duty relief) to try to stay under the trigger.
            rest = [
                (blk, t4, o)
                for blk in range(NBLK)
                for t4 in range(4)
                for o in range(OCH)
            ]
            for blk, t4, o in rest:
                last = (blk, t4) == (1, 3)
                mm2_mm(blk, t4, o, split_copy=last and o >= OCH - 2)
                part = OCH // 4 if last else OCH // 2
                if (o + 1) % part == 0:
                    t0 = blk * BLK
                    lo = (o + 1 - part) * 512
                    nc.sync.dma_start(
                        out[t0 + t4 * 128 : t0 + (t4 + 1) * 128, lo : lo + part * 512],
                        ot_tiles[blk, t4][:, lo : lo + part * 512],
                    )
    _legalize_waits(nc)
    return nc


_NC_CACHE = {}


def _get_nc():
    if "nc" not in _NC_CACHE:
        _NC_CACHE["nc"] = build_bass()
    return _NC_CACHE["nc"]


def _softplus(v):
    return np.logaddexp(0.0, v)


def _host_prep(x, eof_index, noise, A_w, B_w, route_w, noise_w):
    """Routing + gating on host; returns per-core input maps."""
    x = np.asarray(x, dtype=np.float32)
    eof = np.asarray(eof_index).astype(np.int64)
    noise = np.asarray(noise, dtype=np.float32)
    A_w = np.asarray(A_w, dtype=np.float32)
    B_w = np.asarray(B_w, dtype=np.float32)
    route_w = np.asarray(route_w, dtype=np.float32)
    noise_w = np.asarray(noise_w, dtype=np.float32)

    rows = np.arange(B)
    routing_input = x[rows, eof]                                  # [B, D]
    clean = routing_input @ route_w.T                             # [B, E]
    stddev = _softplus(routing_input @ noise_w.T) + NOISE_EPS
    logits = clean + noise * stddev
    top_idx = np.argsort(-logits, axis=-1, kind="stable")[:, :TOPK]
    top_vals = np.take_along_axis(logits, top_idx, axis=-1)
    m = top_vals.max(axis=-1, keepdims=True)
    ex = np.exp(top_vals - m)
    top_gates = (ex / ex.sum(axis=-1, keepdims=True)).astype(np.float32)
    gates = np.zeros((B, E), np.float32)
    np.put_along_axis(gates, top_idx, top_gates, axis=-1)

    # Beff2[b] = SCALING * sum_e gates[b,e] * B_w[e]   -> [B, OUT, R]
    beff2 = SCALING * np.einsum("be,eor->bor", gates, B_w)

    # AT layout [128, DCH, R]: at[p, c, r] = A_w[r, c*128+p]
    at = np.ascontiguousarray(
        A_w.reshape(R, DCH, 128).transpose(2, 1, 0).reshape(128, DCH * R)
    ).astype(BF16)

    in_maps = []
    for i in range(N_CORES):
        b = i * TOK // S
        t0 = i * TOK - b * S
        xc = x[b, t0 : t0 + TOK, :]                               # [TOK, D]
        # [128, NBLK, DCH, BLK]: xT[p, blk, c, t] = xc[blk*BLK+t, c*128+p]
        xT_i = np.ascontiguousarray(
            xc.reshape(NBLK, BLK, DCH, 128).transpose(3, 0, 2, 1)
        ).astype(BF16).reshape(128, NBLK, G, CPG * BLK)
        b2t_i = np.ascontiguousarray(beff2[b].T).astype(BF16)     # [R, OUT]
        in_maps.append({"xT": xT_i, "AT": at, "B2T": b2t_i})
    return in_maps


def _run(in_maps, trace=False, **kw):
    nc = _get_nc()
    return run_bass_kernel_spmd(
        nc, in_maps, core_ids=list(range(N_CORES)), trace=trace, **kw
    )


def kernel(x, eof_index, noise, A_w, B_w, route_w, noise_w):
    in_maps = _host_prep(x, eof_index, noise, A_w, B_w, route_w, noise_w)
    res = _run(in_maps)
    out = np.empty((B, S, OUT), np.float32)
    for i in range(N_CORES):
        b = i * TOK // S
        t0 = i * TOK - b * S
        out[b, t0 : t0 + TOK, :] = np.asarray(res.results[i]["out"]).astype(
            np.float32
        )
    return out
